# revision 2
# baseline (speedup 1.0000x reference)
"""Trainium2 Bass kernel for nn_AttentionLayer (ragged graph attention).

Math (reference, HEADS=1, one query per graph):
  gene' = relu(gene @ fc0_w + fc0_b)            [B,768]
  dense, mask = to_dense_batch(x, batch)        [B,128,768]
  For branch q in {0: gene', 1: bionic'} with weights (wq,wk,wv,wo,biases):
    energy_b = (q_b wq + bq) @ (dense_b wk + bk)^T / sqrt(768)
             = (dense_b @ u_b + c_b) / sqrt(768)
       with u_b = (q_b wq + bq) @ wk^T  (768-vector),  c_b = (q_b wq + bq)·bk
    attn = softmax(masked energy)
    out_b = (attn_b @ dense_b) @ wv @ wo + bv @ wo + bo
  result = out0 + out1                          [B,768]

So the [B*128,768]x[768,768] K/V projections collapse into per-graph
matvecs against dense — the kernel is memory-bound on reading x once.

Distribution: data-parallel over graphs, 32 graphs per core, weights
replicated, no collectives. Host densifies the ragged batch (pure data
marshaling), device does all matrix compute in bf16 (PSUM accumulates
f32), softmax in f32.
"""

import math
import sys
import types

import numpy as np
import ml_dtypes

BF16 = ml_dtypes.bfloat16

B = 256
NPG = 128
HID = 768
DG = 512
NCORES = 8
G = B // NCORES          # graphs per core = 32
R = 2 * G                # interleaved (graph, branch) rows = 64
NCH = HID // 128         # 6 feature chunks
KCF = DG // 128          # 4 fc contraction chunks
SCALE = float(np.sqrt(np.float32(HID)))
NEG = -1e10


def _install_ntff_hook():
    """The agent image's antenv lacks axon_hooks; recreate it so that
    trace=True profiling works (and BASS_TRACE doesn't crash)."""
    try:
        if "antenv.axon_hooks" in sys.modules:
            return
        mod = types.ModuleType("antenv.axon_hooks")
        _h = [None]
        mod.set_axon_ntff_profile_hook = lambda hook: _h.__setitem__(0, hook)
        mod.get_axon_ntff_profile_hook = lambda: _h[0]
        sys.modules["antenv.axon_hooks"] = mod
        import antenv

        antenv.axon_hooks = mod
        from trn_agent_boot.trn_boot import _ntff_profile_via_ctypes

        hook = _ntff_profile_via_ctypes("/opt/axon/libaxon_pjrt.so")
        if hook is not None:
            mod.set_axon_ntff_profile_hook(hook)
    except Exception:
        pass


_NC = None


def _build_nc():
    import concourse.mybir as mybir
    from concourse import bacc, tile
    from concourse import masks

    f32 = mybir.dt.float32
    ct = mybir.dt.bfloat16
    Relu = mybir.ActivationFunctionType.Relu
    Iden = mybir.ActivationFunctionType.Identity
    Copy = mybir.ActivationFunctionType.Copy
    Exp = mybir.ActivationFunctionType.Exp
    AX = mybir.AxisListType.X

    nc = bacc.Bacc("TRN2", target_bir_lowering=False, debug=False)

    d_dense = nc.dram_tensor("dense", [G * NPG, HID], ct, kind="ExternalInput")
    d_mask = nc.dram_tensor("maskadd", [R, NPG], f32, kind="ExternalInput")
    d_geneT = nc.dram_tensor("geneT", [DG, G], ct, kind="ExternalInput")
    d_bionT = nc.dram_tensor("bionicT", [DG, G], ct, kind="ExternalInput")
    dw = {}
    for nm in ("fc0_w", "fc1_w"):
        dw[nm] = nc.dram_tensor(nm, [DG, HID], ct, kind="ExternalInput")
    for nm in ("wq0", "wq1", "wk0T", "wk1T", "wv0", "wv1", "wo0", "wo1"):
        dw[nm] = nc.dram_tensor(nm, [HID, HID], ct, kind="ExternalInput")
    db = {}
    for nm in ("fc0_b", "fc1_b", "bq0", "bq1", "bv0", "bv1"):
        db[nm] = nc.dram_tensor(nm, [HID], f32, kind="ExternalInput")
    d_qk0 = nc.dram_tensor("qk0", [HID], ct, kind="ExternalInput")
    d_qk1 = nc.dram_tensor("qk1", [HID], ct, kind="ExternalInput")
    d_cvec = nc.dram_tensor("cvec", [HID], ct, kind="ExternalInput")
    d_out = nc.dram_tensor("out", [G, HID], f32, kind="ExternalOutput")

    with tile.TileContext(nc) as tc:
        with (
            tc.tile_pool(name="const", bufs=1) as cst,
            tc.tile_pool(name="wts", bufs=1) as wts,
            tc.tile_pool(name="dense", bufs=1) as dns,
            tc.tile_pool(name="work", bufs=1) as wrk,
            tc.tile_pool(name="dT", bufs=3) as dtp,
            tc.tile_pool(name="ps", bufs=1, space="PSUM") as ps,
        ):
            # ---- constants / small inputs ----
            ident_ct = cst.tile([128, 128], ct)
            masks.make_identity(nc, ident_ct[:])
            ident_f32 = cst.tile([128, 128], f32)
            masks.make_identity(nc, ident_f32[:])
            ones_ct = cst.tile([1, 128], ct)
            nc.vector.memset(ones_ct[:], 1.0)

            mask_sb = cst.tile([R, NPG], f32)
            nc.sync.dma_start(mask_sb[:], d_mask[:])
            bias_sb = {}
            for nm in ("fc0_b", "fc1_b", "bq0", "bq1", "bv0", "bv1"):
                t = cst.tile([128, NCH], f32, tag=nm)
                nc.sync.dma_start(t[:], db[nm].rearrange("(c p) -> p c", p=128))
                bias_sb[nm] = t
            qk0_sb = cst.tile([128, NCH], ct)
            nc.sync.dma_start(qk0_sb[:], d_qk0.rearrange("(c p) -> p c", p=128))
            qk1_sb = cst.tile([128, NCH], ct)
            nc.sync.dma_start(qk1_sb[:], d_qk1.rearrange("(c p) -> p c", p=128))
            cvec_sb = cst.tile([1, HID], ct)
            nc.sync.dma_start(cvec_sb[:], d_cvec.rearrange("(o f) -> o f", o=1))
            geneT_sb = cst.tile([128, KCF, G], ct)
            nc.sync.dma_start(geneT_sb[:], d_geneT.rearrange("(c p) g -> p c g", p=128))
            bionT_sb = cst.tile([128, KCF, G], ct)
            nc.sync.dma_start(bionT_sb[:], d_bionT.rearrange("(c p) g -> p c g", p=128))

            # ---- phase A weights, then dense, then phase C weights ----
            w_sb = {}
            for nm in ("fc0_w", "fc1_w"):
                t = wts.tile([128, KCF, HID], ct, tag=nm)
                nc.sync.dma_start(t[:], dw[nm].rearrange("(c p) n -> p c n", p=128))
                w_sb[nm] = t
            for nm in ("wq0", "wq1", "wk0T", "wk1T"):
                t = wts.tile([128, NCH, HID], ct, tag=nm)
                nc.sync.dma_start(t[:], dw[nm].rearrange("(c p) n -> p c n", p=128))
                w_sb[nm] = t

            dense_sb = []
            for i in range(8):
                t = dns.tile([128, 4, HID], ct, tag=f"dense{i}")
                nc.sync.dma_start(
                    t[:],
                    d_dense.rearrange("(t g p) f -> t p g f", g=4, p=128)[i],
                )
                dense_sb.append(t)

            for nm in ("wv0", "wv1", "wo0", "wo1"):
                t = wts.tile([128, NCH, HID], ct, tag=nm)
                nc.sync.dma_start(t[:], dw[nm].rearrange("(c p) n -> p c n", p=128))
                w_sb[nm] = t

            def dview(b):
                return dense_sb[b // 4][:, b % 4, :]

            # ---- phase A: activationsT chains (features on partitions) ----
            def chain(dst, w, kcn, rhs, bias=None, func=Iden):
                for mc in range(NCH):
                    pt = ps.tile([128, G], f32, tag="mm32")
                    for kc in range(kcn):
                        nc.tensor.matmul(
                            pt[:],
                            w[:, kc, mc * 128:(mc + 1) * 128],
                            rhs[:, kc, :],
                            start=(kc == 0),
                            stop=(kc == kcn - 1),
                        )
                    bb = bias[:, mc:mc + 1] if bias is not None else 0.0
                    nc.scalar.activation(dst[:, mc, :], pt[:], func, bias=bb, scale=1.0)

            gene_pT = wrk.tile([128, NCH, G], ct)
            chain(gene_pT, w_sb["fc0_w"], KCF, geneT_sb, bias_sb["fc0_b"], Relu)
            bion_pT = wrk.tile([128, NCH, G], ct)
            chain(bion_pT, w_sb["fc1_w"], KCF, bionT_sb, bias_sb["fc1_b"], Relu)
            y0T = wrk.tile([128, NCH, G], ct)
            chain(y0T, w_sb["wq0"], NCH, gene_pT, bias_sb["bq0"], Iden)
            y1T = wrk.tile([128, NCH, G], ct)
            chain(y1T, w_sb["wq1"], NCH, bion_pT, bias_sb["bq1"], Iden)

            # u vectors, interleaved (graph-major, branch minor) [128, NCH, R]
            uT = wrk.tile([128, NCH, R], ct)
            for mc in range(NCH):
                usl = uT[:, mc, :].rearrange("p (g q) -> p q g", q=2)
                for q, (wk, yT) in enumerate((("wk0T", y0T), ("wk1T", y1T))):
                    pt = ps.tile([128, G], f32, tag="mm32")
                    for kc in range(NCH):
                        nc.tensor.matmul(
                            pt[:],
                            w_sb[wk][:, kc, mc * 128:(mc + 1) * 128],
                            yT[:, kc, :],
                            start=(kc == 0),
                            stop=(kc == NCH - 1),
                        )
                    nc.vector.tensor_copy(usl[:, q, :], pt[:])

            # c scalars: c_q[b] = q'_b · qk_q  -> [1, R] interleaved
            c_int = wrk.tile([1, R], ct)
            csl = c_int[0:1, :].rearrange("p (g q) -> p q g", q=2)
            for q, (qk, aT) in enumerate(((qk0_sb, gene_pT), (qk1_sb, bion_pT))):
                pt = ps.tile([1, G], f32, tag="mm32")
                for kc in range(NCH):
                    nc.tensor.matmul(
                        pt[:],
                        qk[:, kc:kc + 1],
                        aT[:, kc, :],
                        start=(kc == 0),
                        stop=(kc == NCH - 1),
                    )
                nc.vector.tensor_copy(csl[:, q, :], pt[:])

            # ---- phase B: stream graphs; e_ps[node, 2b+q] = dense_b @ u + c ----
            e_ps = ps.tile([128, R], f32, tag="eacc")
            for b in range(G):
                dv = dview(b)
                dT = dtp.tile([128, HID], ct, tag="dT")
                for pair in range(3):
                    tp = ps.tile([128, 256], ct, tag="tp")
                    for k in range(2):
                        mc = pair * 2 + k
                        nc.tensor.transpose(
                            tp[:, k * 128:(k + 1) * 128],
                            dv[:, mc * 128:(mc + 1) * 128],
                            ident_ct[:],
                        )
                    if pair == 2:
                        nc.scalar.copy(dT[:, pair * 256:(pair + 1) * 256], tp[:])
                    else:
                        nc.vector.tensor_copy(dT[:, pair * 256:(pair + 1) * 256], tp[:])
                for mc in range(NCH):
                    nc.tensor.matmul(
                        e_ps[:, 2 * b:2 * b + 2],
                        dT[:, mc * 128:(mc + 1) * 128],
                        uT[:, mc, 2 * b:2 * b + 2],
                        start=(mc == 0),
                        stop=False,
                    )
                nc.tensor.matmul(
                    e_ps[:, 2 * b:2 * b + 2],
                    ones_ct[:],
                    c_int[0:1, 2 * b:2 * b + 2],
                    start=False,
                    stop=True,
                )

            # ---- softmax over nodes (batched, rows = 2b+q) ----
            e_sb = wrk.tile([128, R], f32)
            nc.scalar.activation(e_sb[:], e_ps[:], Copy, bias=0.0, scale=1.0 / SCALE)
            etp = ps.tile([R, NPG], f32, tag="tp")
            nc.tensor.transpose(etp[:], e_sb[:], ident_f32[:])
            em = wrk.tile([R, NPG], f32)
            nc.vector.tensor_add(em[:], etp[:], mask_sb[:])
            nmax = wrk.tile([R, 1], f32)
            nc.vector.reduce_max(nmax[:], em[:], axis=AX, negate=True)
            p_sb = wrk.tile([R, NPG], f32)
            rsum = wrk.tile([R, 1], f32)
            nc.scalar.activation(
                p_sb[:], em[:], Exp, bias=nmax[:, 0:1], scale=1.0, accum_out=rsum[:]
            )
            rinv = wrk.tile([R, 1], f32)
            nc.vector.reciprocal(rinv[:], rsum[:])
            attn = wrk.tile([R, NPG], ct)
            nc.vector.tensor_scalar_mul(attn[:], p_sb[:], rinv[:, 0:1])
            atp = ps.tile([128, R], ct, tag="tp")
            nc.tensor.transpose(atp[:], attn[:], ident_ct[0:R, 0:R])
            attnT = wrk.tile([128, R], ct)
            nc.vector.tensor_copy(attnT[:], atp[:])

            # ---- h: h_ps[feat_chunk, mc*R + 2b+q] = attn_{b,q} @ dense_b ----
            h_ps = ps.tile([128, NCH * R], f32, tag="hacc")
            for b in range(G):
                dv = dview(b)
                for mc in range(NCH):
                    nc.tensor.matmul(
                        h_ps[:, mc * R + 2 * b: mc * R + 2 * b + 2],
                        dv[:, mc * 128:(mc + 1) * 128],
                        attnT[:, 2 * b:2 * b + 2],
                        start=True,
                        stop=True,
                    )
            h0T = wrk.tile([128, NCH, G], ct)
            h1T = wrk.tile([128, NCH, G], ct)
            for mc in range(NCH):
                hsl = h_ps[:, mc * R:(mc + 1) * R].rearrange("p (g q) -> p q g", q=2)
                nc.vector.tensor_copy(h0T[:, mc, :], hsl[:, 0, :])
                nc.scalar.copy(h1T[:, mc, :], hsl[:, 1, :])

            # ---- phase C: out = (h0 wv0 + bv0) wo0 + (h1 wv1 + bv1) wo1 + cvec ----
            z0T = wrk.tile([128, NCH, G], ct)
            chain(z0T, w_sb["wv0"], NCH, h0T, bias_sb["bv0"], Iden)
            z1T = wrk.tile([128, NCH, G], ct)
            chain(z1T, w_sb["wv1"], NCH, h1T, bias_sb["bv1"], Iden)

            out_sb = wrk.tile([G, HID], f32)
            for half in range(2):
                nsl = slice(half * 384, (half + 1) * 384)
                po = ps.tile([G, 384], f32, tag="tp")
                for kc in range(NCH):
                    nc.tensor.matmul(
                        po[:], z0T[:, kc, :], w_sb["wo0"][:, kc, nsl],
                        start=(kc == 0), stop=False,
                    )
                for kc in range(NCH):
                    nc.tensor.matmul(
                        po[:], z1T[:, kc, :], w_sb["wo1"][:, kc, nsl],
                        start=False, stop=False,
                    )
                nc.tensor.matmul(
                    po[:], ones_ct[0:1, 0:G], cvec_sb[0:1, nsl],
                    start=False, stop=True,
                )
                nc.vector.tensor_copy(out_sb[:, nsl], po[:])
            nc.sync.dma_start(d_out[:], out_sb[:])

    nc.compile()
    return nc


def _get_nc():
    global _NC
    if _NC is None:
        _NC = _build_nc()
    return _NC


def _prep_in_maps(inputs):
    x = np.asarray(inputs["x"], dtype=np.float32)
    batch = np.asarray(inputs["batch"]).astype(np.int64)
    gene = np.asarray(inputs["gene"], dtype=np.float32)
    bionic = np.asarray(inputs["bionic"], dtype=np.float32)
    N = x.shape[0]

    counts = np.bincount(batch, minlength=B)[:B]
    starts = np.zeros(B, np.int64)
    np.cumsum(counts[:-1], out=starts[1:])
    pos = np.arange(N, dtype=np.int64) - starts[batch]
    ok = pos < NPG
    dense = np.zeros((B, NPG, HID), np.float32)
    dense[batch[ok], pos[ok]] = x[ok]
    dense_bf = dense.astype(BF16)
    mask = np.where(
        np.arange(NPG)[None, :] < np.minimum(counts, NPG)[:, None], 0.0, NEG
    ).astype(np.float32)

    f32 = lambda k: np.asarray(inputs[k], dtype=np.float32)
    w = {
        "fc0_w": f32("fc0_w").astype(BF16),
        "fc1_w": f32("fc1_w").astype(BF16),
        "wq0": f32("a0_wq").astype(BF16),
        "wq1": f32("a1_wq").astype(BF16),
        "wk0T": np.ascontiguousarray(f32("a0_wk").T).astype(BF16),
        "wk1T": np.ascontiguousarray(f32("a1_wk").T).astype(BF16),
        "wv0": f32("a0_wv").astype(BF16),
        "wv1": f32("a1_wv").astype(BF16),
        "wo0": f32("a0_wo").astype(BF16),
        "wo1": f32("a1_wo").astype(BF16),
        "fc0_b": f32("fc0_b"),
        "fc1_b": f32("fc1_b"),
        "bq0": f32("a0_bq"),
        "bq1": f32("a1_bq"),
        "bv0": f32("a0_bv"),
        "bv1": f32("a1_bv"),
        "qk0": (f32("a0_wq") @ f32("a0_bk")).astype(BF16),
        "qk1": (f32("a1_wq") @ f32("a1_bk")).astype(BF16),
        "cvec": (f32("a0_bo") + f32("a1_bo")).astype(BF16),
    }
    s0 = float(f32("a0_bq") @ f32("a0_bk"))
    s1 = float(f32("a1_bq") @ f32("a1_bk"))

    in_maps = []
    for j in range(NCORES):
        g0, g1 = j * G, (j + 1) * G
        m = np.repeat(mask[g0:g1], 2, axis=0)
        m[0::2] += s0 / SCALE
        m[1::2] += s1 / SCALE
        im = {
            "dense": np.ascontiguousarray(dense_bf[g0:g1].reshape(G * NPG, HID)),
            "maskadd": np.ascontiguousarray(m),
            "geneT": np.ascontiguousarray(gene[g0:g1].T).astype(BF16),
            "bionicT": np.ascontiguousarray(bionic[g0:g1].T).astype(BF16),
        }
        im.update(w)
        in_maps.append(im)
    return in_maps


def run(inputs, trace=False):
    _install_ntff_hook()
    from concourse.bass_utils import run_bass_kernel_spmd

    nc = _get_nc()
    in_maps = _prep_in_maps(inputs)
    res = run_bass_kernel_spmd(
        nc, in_maps, core_ids=list(range(NCORES)), trace=trace
    )
    out = np.concatenate([res.results[i]["out"] for i in range(NCORES)], axis=0)
    return out.astype(np.float32), res


def kernel(**inputs):
    return run(inputs, trace=False)[0]


# revision 3
# speedup vs baseline: 1.4714x; 1.4714x over previous
"""Trainium2 Bass kernel for nn_AttentionLayer (ragged graph attention).

Math (reference, HEADS=1, one query per graph):
  gene' = relu(gene @ fc0_w + fc0_b)            [B,768]
  dense, mask = to_dense_batch(x, batch)        [B,128,768]
  For branch q in {0: gene', 1: bionic'} with weights (wq,wk,wv,wo,biases):
    energy_b = (q_b wq + bq) @ (dense_b wk + bk)^T / sqrt(768)
             = (dense_b @ u_b + c_b) / sqrt(768)
       with u_b = (q_b wq + bq) @ wk^T  (768-vector),  c_b = (q_b wq + bq)·bk
    attn = softmax(masked energy)
    out_b = (attn_b @ dense_b) @ wv @ wo + bv @ wo + bo
  result = out0 + out1                          [B,768]

So the [B*128,768]x[768,768] K/V projections collapse into per-graph
matvecs against dense — the kernel is memory-bound on reading x once.

Distribution: data-parallel over graphs, 32 graphs per core, weights
replicated, no collectives. Host densifies the ragged batch (pure data
marshaling), device does all matrix compute in bf16 (PSUM accumulates
f32), softmax in f32.
"""

import math
import sys
import types

import numpy as np
import ml_dtypes

BF16 = ml_dtypes.bfloat16

B = 256
NPG = 128
HID = 768
DG = 512
NCORES = 8
G = B // NCORES          # graphs per core = 32
R = 2 * G                # interleaved (graph, branch) rows = 64
NCH = HID // 128         # 6 feature chunks
KCF = DG // 128          # 4 fc contraction chunks
SCALE = float(np.sqrt(np.float32(HID)))
NEG = -1e10


def _install_ntff_hook():
    """The agent image's antenv lacks axon_hooks; recreate it so that
    trace=True profiling works (and BASS_TRACE doesn't crash)."""
    try:
        if "antenv.axon_hooks" in sys.modules:
            return
        mod = types.ModuleType("antenv.axon_hooks")
        _h = [None]
        mod.set_axon_ntff_profile_hook = lambda hook: _h.__setitem__(0, hook)
        mod.get_axon_ntff_profile_hook = lambda: _h[0]
        sys.modules["antenv.axon_hooks"] = mod
        import antenv

        antenv.axon_hooks = mod
        from trn_agent_boot.trn_boot import _ntff_profile_via_ctypes

        hook = _ntff_profile_via_ctypes("/opt/axon/libaxon_pjrt.so")
        if hook is not None:
            mod.set_axon_ntff_profile_hook(hook)
    except Exception:
        pass


_NC = None


def _build_nc():
    import concourse.mybir as mybir
    from concourse import bacc, tile
    from concourse import masks

    f32 = mybir.dt.float32
    ct = mybir.dt.bfloat16
    Relu = mybir.ActivationFunctionType.Relu
    Iden = mybir.ActivationFunctionType.Identity
    Copy = mybir.ActivationFunctionType.Copy
    Exp = mybir.ActivationFunctionType.Exp
    AX = mybir.AxisListType.X

    nc = bacc.Bacc("TRN2", target_bir_lowering=False, debug=False)

    d_dense = nc.dram_tensor("dense", [G * NPG, HID], ct, kind="ExternalInput")
    d_denseT = nc.dram_tensor("denseT", [G * HID, NPG], ct, kind="ExternalInput")
    d_mask = nc.dram_tensor("maskadd", [R, NPG], f32, kind="ExternalInput")
    d_geneT = nc.dram_tensor("geneT", [DG, G], ct, kind="ExternalInput")
    d_bionT = nc.dram_tensor("bionicT", [DG, G], ct, kind="ExternalInput")
    dw = {}
    for nm in ("fc0_w", "fc1_w"):
        dw[nm] = nc.dram_tensor(nm, [DG, HID], ct, kind="ExternalInput")
    for nm in ("wq0", "wq1", "wk0T", "wk1T", "wv0", "wv1", "wo0", "wo1"):
        dw[nm] = nc.dram_tensor(nm, [HID, HID], ct, kind="ExternalInput")
    db = {}
    for nm in ("fc0_b", "fc1_b", "bq0", "bq1", "bv0", "bv1"):
        db[nm] = nc.dram_tensor(nm, [HID], f32, kind="ExternalInput")
    d_qk0 = nc.dram_tensor("qk0", [HID], ct, kind="ExternalInput")
    d_qk1 = nc.dram_tensor("qk1", [HID], ct, kind="ExternalInput")
    d_cvec = nc.dram_tensor("cvec", [HID], ct, kind="ExternalInput")
    d_out = nc.dram_tensor("out", [G, HID], f32, kind="ExternalOutput")

    with tile.TileContext(nc) as tc:
        with (
            tc.tile_pool(name="const", bufs=1) as cst,
            tc.tile_pool(name="wts", bufs=1) as wts,
            tc.tile_pool(name="dense", bufs=1) as dns,
            tc.tile_pool(name="work", bufs=1) as wrk,
            tc.tile_pool(name="dTs", bufs=4) as dtp,
            tc.tile_pool(name="ps", bufs=1, space="PSUM") as ps,
        ):
            # ---- constants / small inputs ----
            ident_ct = cst.tile([128, 128], ct)
            masks.make_identity(nc, ident_ct[:])
            ident_f32 = cst.tile([128, 128], f32)
            masks.make_identity(nc, ident_f32[:])
            ones_ct = cst.tile([1, 128], ct)
            nc.vector.memset(ones_ct[:], 1.0)

            mask_sb = cst.tile([R, NPG], f32)
            nc.sync.dma_start(mask_sb[:], d_mask[:])
            bias_sb = {}
            for nm in ("fc0_b", "fc1_b", "bq0", "bq1", "bv0", "bv1"):
                t = cst.tile([128, NCH], f32, tag=nm)
                nc.sync.dma_start(t[:], db[nm].rearrange("(c p) -> p c", p=128))
                bias_sb[nm] = t
            qk0_sb = cst.tile([128, NCH], ct)
            nc.sync.dma_start(qk0_sb[:], d_qk0.rearrange("(c p) -> p c", p=128))
            qk1_sb = cst.tile([128, NCH], ct)
            nc.sync.dma_start(qk1_sb[:], d_qk1.rearrange("(c p) -> p c", p=128))
            cvec_sb = cst.tile([1, HID], ct)
            nc.sync.dma_start(cvec_sb[:], d_cvec.rearrange("(o f) -> o f", o=1))
            geneT_sb = cst.tile([128, KCF, G], ct)
            nc.sync.dma_start(geneT_sb[:], d_geneT.rearrange("(c p) g -> p c g", p=128))
            bionT_sb = cst.tile([128, KCF, G], ct)
            nc.sync.dma_start(bionT_sb[:], d_bionT.rearrange("(c p) g -> p c g", p=128))

            # ---- phase A weights, then dense, then phase C weights ----
            w_sb = {}
            for nm in ("fc0_w", "fc1_w"):
                t = wts.tile([128, KCF, HID], ct, tag=nm)
                nc.sync.dma_start(t[:], dw[nm].rearrange("(c p) n -> p c n", p=128))
                w_sb[nm] = t
            for nm in ("wq0", "wq1", "wk0T", "wk1T"):
                t = wts.tile([128, NCH, HID], ct, tag=nm)
                nc.sync.dma_start(t[:], dw[nm].rearrange("(c p) n -> p c n", p=128))
                w_sb[nm] = t

            dense_sb = []
            for i in range(8):
                t = dns.tile([128, 4, HID], ct, tag=f"dense{i}")
                nc.sync.dma_start(
                    t[:],
                    d_dense.rearrange("(t g p) f -> t p g f", g=4, p=128)[i],
                )
                dense_sb.append(t)

            for nm in ("wv0", "wv1", "wo0", "wo1"):
                t = wts.tile([128, NCH, HID], ct, tag=nm)
                nc.sync.dma_start(t[:], dw[nm].rearrange("(c p) n -> p c n", p=128))
                w_sb[nm] = t

            def dview(b):
                return dense_sb[b // 4][:, b % 4, :]

            # ---- phase A: activationsT chains (features on partitions) ----
            def chain(dst, w, kcn, rhs, bias=None, func=Iden):
                for mc in range(NCH):
                    pt = ps.tile([128, G], f32, tag="mm32")
                    for kc in range(kcn):
                        nc.tensor.matmul(
                            pt[:],
                            w[:, kc, mc * 128:(mc + 1) * 128],
                            rhs[:, kc, :],
                            start=(kc == 0),
                            stop=(kc == kcn - 1),
                        )
                    bb = bias[:, mc:mc + 1] if bias is not None else 0.0
                    nc.scalar.activation(dst[:, mc, :], pt[:], func, bias=bb, scale=1.0)

            gene_pT = wrk.tile([128, NCH, G], ct)
            chain(gene_pT, w_sb["fc0_w"], KCF, geneT_sb, bias_sb["fc0_b"], Relu)
            bion_pT = wrk.tile([128, NCH, G], ct)
            chain(bion_pT, w_sb["fc1_w"], KCF, bionT_sb, bias_sb["fc1_b"], Relu)
            y0T = wrk.tile([128, NCH, G], ct)
            chain(y0T, w_sb["wq0"], NCH, gene_pT, bias_sb["bq0"], Iden)
            y1T = wrk.tile([128, NCH, G], ct)
            chain(y1T, w_sb["wq1"], NCH, bion_pT, bias_sb["bq1"], Iden)

            # u vectors, interleaved (graph-major, branch minor) [128, NCH, R]
            uT = wrk.tile([128, NCH, R], ct)
            for mc in range(NCH):
                usl = uT[:, mc, :].rearrange("p (g q) -> p q g", q=2)
                for q, (wk, yT) in enumerate((("wk0T", y0T), ("wk1T", y1T))):
                    pt = ps.tile([128, G], f32, tag="mm32")
                    for kc in range(NCH):
                        nc.tensor.matmul(
                            pt[:],
                            w_sb[wk][:, kc, mc * 128:(mc + 1) * 128],
                            yT[:, kc, :],
                            start=(kc == 0),
                            stop=(kc == NCH - 1),
                        )
                    nc.vector.tensor_copy(usl[:, q, :], pt[:])

            # c scalars: c_q[b] = q'_b · qk_q  -> [1, R] interleaved
            c_int = wrk.tile([1, R], ct)
            csl = c_int[0:1, :].rearrange("p (g q) -> p q g", q=2)
            for q, (qk, aT) in enumerate(((qk0_sb, gene_pT), (qk1_sb, bion_pT))):
                pt = ps.tile([1, G], f32, tag="mm32")
                for kc in range(NCH):
                    nc.tensor.matmul(
                        pt[:],
                        qk[:, kc:kc + 1],
                        aT[:, kc, :],
                        start=(kc == 0),
                        stop=(kc == NCH - 1),
                    )
                nc.vector.tensor_copy(csl[:, q, :], pt[:])

            # ---- phase B: e_ps[node, 2b+q] = dense_b @ u_bq + c_bq ----
            # c broadcast first (start=True clears the whole tile), then
            # per-graph accumulating matvecs against host-transposed denseT.
            e_ps = ps.tile([128, R], f32, tag="eacc")
            nc.tensor.matmul(
                e_ps[:], ones_ct[:], c_int[:], start=True, stop=False,
                skip_group_check=True,
            )
            for i in range(8):
                dTt = dtp.tile([128, 4, NCH, 128], ct, tag="dTs")
                nc.gpsimd.dma_start(
                    dTt[:],
                    d_denseT.rearrange("(t g c p) n -> t p g c n", g=4, c=NCH, p=128)[i],
                )
                for g in range(4):
                    b = i * 4 + g
                    for mc in range(NCH):
                        nc.tensor.matmul(
                            e_ps[:, 2 * b:2 * b + 2],
                            dTt[:, g, mc, :],
                            uT[:, mc, 2 * b:2 * b + 2],
                            start=False,
                            stop=(b == G - 1 and mc == NCH - 1),
                            skip_group_check=True,
                        )

            # ---- softmax over nodes (batched, rows = 2b+q) ----
            e_sb = wrk.tile([128, R], f32)
            nc.scalar.activation(e_sb[:], e_ps[:], Copy, bias=0.0, scale=1.0 / SCALE)
            etp = ps.tile([R, NPG], f32, tag="tp")
            nc.tensor.transpose(etp[:], e_sb[:], ident_f32[:])
            em = wrk.tile([R, NPG], f32)
            nc.vector.tensor_add(em[:], etp[:], mask_sb[:])
            nmax = wrk.tile([R, 1], f32)
            nc.vector.reduce_max(nmax[:], em[:], axis=AX, negate=True)
            p_sb = wrk.tile([R, NPG], f32)
            rsum = wrk.tile([R, 1], f32)
            nc.scalar.activation(
                p_sb[:], em[:], Exp, bias=nmax[:, 0:1], scale=1.0, accum_out=rsum[:]
            )
            rinv = wrk.tile([R, 1], f32)
            nc.vector.reciprocal(rinv[:], rsum[:])
            attn = wrk.tile([R, NPG], ct)
            nc.vector.tensor_scalar_mul(attn[:], p_sb[:], rinv[:, 0:1])
            atp = ps.tile([128, R], ct, tag="tp")
            nc.tensor.transpose(atp[:], attn[:], ident_ct[0:R, 0:R])
            attnT = wrk.tile([128, R], ct)
            nc.vector.tensor_copy(attnT[:], atp[:])

            # ---- h: h_ps[feat_chunk, mc*R + 2b+q] = attn_{b,q} @ dense_b ----
            h_ps = ps.tile([128, NCH * R], f32, tag="hacc")
            for b in range(G):
                dv = dview(b)
                for mc in range(NCH):
                    nc.tensor.matmul(
                        h_ps[:, mc * R + 2 * b: mc * R + 2 * b + 2],
                        dv[:, mc * 128:(mc + 1) * 128],
                        attnT[:, 2 * b:2 * b + 2],
                        start=True,
                        stop=True,
                    )
            h0T = wrk.tile([128, NCH, G], ct)
            h1T = wrk.tile([128, NCH, G], ct)
            for mc in range(NCH):
                hsl = h_ps[:, mc * R:(mc + 1) * R].rearrange("p (g q) -> p q g", q=2)
                nc.vector.tensor_copy(h0T[:, mc, :], hsl[:, 0, :])
                nc.scalar.copy(h1T[:, mc, :], hsl[:, 1, :])

            # ---- phase C: out = (h0 wv0 + bv0) wo0 + (h1 wv1 + bv1) wo1 + cvec ----
            z0T = wrk.tile([128, NCH, G], ct)
            chain(z0T, w_sb["wv0"], NCH, h0T, bias_sb["bv0"], Iden)
            z1T = wrk.tile([128, NCH, G], ct)
            chain(z1T, w_sb["wv1"], NCH, h1T, bias_sb["bv1"], Iden)

            out_sb = wrk.tile([G, HID], f32)
            for half in range(2):
                nsl = slice(half * 384, (half + 1) * 384)
                po = ps.tile([G, 384], f32, tag="tp")
                for kc in range(NCH):
                    nc.tensor.matmul(
                        po[:], z0T[:, kc, :], w_sb["wo0"][:, kc, nsl],
                        start=(kc == 0), stop=False,
                    )
                for kc in range(NCH):
                    nc.tensor.matmul(
                        po[:], z1T[:, kc, :], w_sb["wo1"][:, kc, nsl],
                        start=False, stop=False,
                    )
                nc.tensor.matmul(
                    po[:], ones_ct[0:1, 0:G], cvec_sb[0:1, nsl],
                    start=False, stop=True,
                )
                nc.vector.tensor_copy(out_sb[:, nsl], po[:])
            nc.sync.dma_start(d_out[:], out_sb[:])

    nc.compile()
    return nc


def _get_nc():
    global _NC
    if _NC is None:
        _NC = _build_nc()
    return _NC


def _prep_in_maps(inputs):
    x = np.asarray(inputs["x"], dtype=np.float32)
    batch = np.asarray(inputs["batch"]).astype(np.int64)
    gene = np.asarray(inputs["gene"], dtype=np.float32)
    bionic = np.asarray(inputs["bionic"], dtype=np.float32)
    N = x.shape[0]

    counts = np.bincount(batch, minlength=B)[:B]
    starts = np.zeros(B, np.int64)
    np.cumsum(counts[:-1], out=starts[1:])
    pos = np.arange(N, dtype=np.int64) - starts[batch]
    ok = pos < NPG
    dense = np.zeros((B, NPG, HID), np.float32)
    dense[batch[ok], pos[ok]] = x[ok]
    dense_bf = dense.astype(BF16)
    mask = np.where(
        np.arange(NPG)[None, :] < np.minimum(counts, NPG)[:, None], 0.0, NEG
    ).astype(np.float32)

    f32 = lambda k: np.asarray(inputs[k], dtype=np.float32)
    w = {
        "fc0_w": f32("fc0_w").astype(BF16),
        "fc1_w": f32("fc1_w").astype(BF16),
        "wq0": f32("a0_wq").astype(BF16),
        "wq1": f32("a1_wq").astype(BF16),
        "wk0T": np.ascontiguousarray(f32("a0_wk").T).astype(BF16),
        "wk1T": np.ascontiguousarray(f32("a1_wk").T).astype(BF16),
        "wv0": f32("a0_wv").astype(BF16),
        "wv1": f32("a1_wv").astype(BF16),
        "wo0": f32("a0_wo").astype(BF16),
        "wo1": f32("a1_wo").astype(BF16),
        "fc0_b": f32("fc0_b"),
        "fc1_b": f32("fc1_b"),
        "bq0": f32("a0_bq"),
        "bq1": f32("a1_bq"),
        "bv0": f32("a0_bv"),
        "bv1": f32("a1_bv"),
        "qk0": (f32("a0_wq") @ f32("a0_bk")).astype(BF16),
        "qk1": (f32("a1_wq") @ f32("a1_bk")).astype(BF16),
        "cvec": (f32("a0_bo") + f32("a1_bo")).astype(BF16),
    }
    s0 = float(f32("a0_bq") @ f32("a0_bk"))
    s1 = float(f32("a1_bq") @ f32("a1_bk"))

    in_maps = []
    for j in range(NCORES):
        g0, g1 = j * G, (j + 1) * G
        m = np.repeat(mask[g0:g1], 2, axis=0)
        m[0::2] += s0 / SCALE
        m[1::2] += s1 / SCALE
        im = {
            "dense": np.ascontiguousarray(dense_bf[g0:g1].reshape(G * NPG, HID)),
            "denseT": np.ascontiguousarray(
                dense_bf[g0:g1].transpose(0, 2, 1)
            ).reshape(G * HID, NPG),
            "maskadd": np.ascontiguousarray(m),
            "geneT": np.ascontiguousarray(gene[g0:g1].T).astype(BF16),
            "bionicT": np.ascontiguousarray(bionic[g0:g1].T).astype(BF16),
        }
        im.update(w)
        in_maps.append(im)
    return in_maps


def run(inputs, trace=False):
    _install_ntff_hook()
    from concourse.bass_utils import run_bass_kernel_spmd

    nc = _get_nc()
    in_maps = _prep_in_maps(inputs)
    res = run_bass_kernel_spmd(
        nc, in_maps, core_ids=list(range(NCORES)), trace=trace
    )
    out = np.concatenate([res.results[i]["out"] for i in range(NCORES)], axis=0)
    return out.astype(np.float32), res


def kernel(**inputs):
    return run(inputs, trace=False)[0]


# revision 5
# speedup vs baseline: 2.1035x; 1.4296x over previous
"""Trainium2 Bass kernel for nn_AttentionLayer (ragged graph attention).

Math (reference, HEADS=1, one query per graph):
  gene' = relu(gene @ fc0_w + fc0_b)            [B,768]
  dense, mask = to_dense_batch(x, batch)        [B,128,768]
  For branch q in {0: gene', 1: bionic'} with weights (wq,wk,wv,wo,biases):
    energy_b = (q_b wq + bq) @ (dense_b wk + bk)^T / sqrt(768)
             = (dense_b @ u_b + c_b) / sqrt(768)
       with u_b = (q_b wq + bq) @ wk^T  (768-vector),  c_b = (q_b wq + bq)·bk
    attn = softmax(masked energy)
    out_b = (attn_b @ dense_b) @ wv @ wo + bv @ wo + bo
  result = out0 + out1                          [B,768]

So the [B*128,768]x[768,768] K/V projections collapse into per-graph
matvecs against dense — the kernel is memory-bound on reading x once.

Distribution: data-parallel over graphs, 32 graphs per core, weights
replicated, no collectives. The host densifies the ragged batch and
pre-swizzles every device input into partition-major layout so each
DMA is a plain [128 x contiguous] pattern (128 descriptors — the HWDGE
descriptor-generation path is the scarce resource, not bandwidth).
Device compute in bf16 (PSUM accumulates f32), softmax in f32.

Row convention: interleaved r = 2*b + q (graph-major, branch minor).
"""

import sys
import types

import numpy as np
import ml_dtypes

BF16 = ml_dtypes.bfloat16

B = 256
NPG = 128
HID = 768
DG = 512
NCORES = 8
G = B // NCORES          # graphs per core = 32
R = 2 * G                # interleaved (graph, branch) rows = 64
NCH = HID // 128         # 6 feature chunks
KCF = DG // 128          # 4 fc contraction chunks
SCALE = float(np.sqrt(np.float32(HID)))
NEG = -1e10

BIAS_NAMES = ("fc0_b", "fc1_b", "bq0", "bq1", "bv0", "bv1")


def _install_ntff_hook():
    """The agent image's antenv lacks axon_hooks; recreate it so that
    trace=True profiling works (and BASS_TRACE doesn't crash)."""
    try:
        if "antenv.axon_hooks" in sys.modules:
            return
        mod = types.ModuleType("antenv.axon_hooks")
        _h = [None]
        mod.set_axon_ntff_profile_hook = lambda hook: _h.__setitem__(0, hook)
        mod.get_axon_ntff_profile_hook = lambda: _h[0]
        sys.modules["antenv.axon_hooks"] = mod
        import antenv

        antenv.axon_hooks = mod
        from trn_agent_boot.trn_boot import _ntff_profile_via_ctypes

        hook = _ntff_profile_via_ctypes("/opt/axon/libaxon_pjrt.so")
        if hook is not None:
            mod.set_axon_ntff_profile_hook(hook)
    except Exception:
        pass


_NC = None


def _build_nc():
    import concourse.mybir as mybir
    from concourse import bacc, tile
    from concourse import masks

    f32 = mybir.dt.float32
    ct = mybir.dt.bfloat16
    Relu = mybir.ActivationFunctionType.Relu
    Iden = mybir.ActivationFunctionType.Identity
    Copy = mybir.ActivationFunctionType.Copy
    Exp = mybir.ActivationFunctionType.Exp
    AX = mybir.AxisListType.X

    nc = bacc.Bacc("TRN2", target_bir_lowering=False, debug=False)

    # All inputs are host-swizzled partition-major: first dim = partition.
    d_dense = nc.dram_tensor("dense", [128, G, HID], ct, kind="ExternalInput")
    d_denseT = nc.dram_tensor("denseT", [128, G, NCH, 128], ct, kind="ExternalInput")
    d_mask = nc.dram_tensor("maskadd", [R, NPG], f32, kind="ExternalInput")
    d_acts = nc.dram_tensor("acts", [128, 2, KCF, G], ct, kind="ExternalInput")
    d_wfc = nc.dram_tensor("wfc", [128, 2, KCF, HID], ct, kind="ExternalInput")
    d_wqk = nc.dram_tensor("wqk", [128, 4, NCH, HID], ct, kind="ExternalInput")
    d_wvo = nc.dram_tensor("wvo", [128, 4, NCH, HID], ct, kind="ExternalInput")
    d_bias = nc.dram_tensor("bias", [128, len(BIAS_NAMES), NCH], f32, kind="ExternalInput")
    d_qks = nc.dram_tensor("qks", [128, 2, NCH], ct, kind="ExternalInput")
    d_cvec = nc.dram_tensor("cvec", [1, HID], ct, kind="ExternalInput")
    d_out = nc.dram_tensor("out", [G, HID], f32, kind="ExternalOutput")

    with tile.TileContext(nc) as tc:
        with (
            tc.tile_pool(name="const", bufs=1) as cst,
            tc.tile_pool(name="wts", bufs=1) as wts,
            tc.tile_pool(name="dense", bufs=1) as dns,
            tc.tile_pool(name="work", bufs=1) as wrk,
            tc.tile_pool(name="ps", bufs=1, space="PSUM") as ps,
        ):
            # ---- constants / small inputs (sync queue) ----
            ident_ct = cst.tile([128, 128], ct)
            masks.make_identity(nc, ident_ct[:])
            ident_f32 = cst.tile([128, 128], f32)
            masks.make_identity(nc, ident_f32[:])
            ones_ct = cst.tile([1, 128], ct)
            nc.vector.memset(ones_ct[:], 1.0)

            mask_sb = cst.tile([R, NPG], f32)
            nc.sync.dma_start(mask_sb[:], d_mask[:])
            acts_sb = cst.tile([128, 2, KCF, G], ct)
            nc.sync.dma_start(acts_sb[:], d_acts[:])
            bias_pk = cst.tile([128, len(BIAS_NAMES), NCH], f32)
            nc.sync.dma_start(bias_pk[:], d_bias[:])
            qks_sb = cst.tile([128, 2, NCH], ct)
            nc.sync.dma_start(qks_sb[:], d_qks[:])
            cvec_sb = cst.tile([1, HID], ct)
            nc.sync.dma_start(cvec_sb[:], d_cvec[:])
            bias_sb = {nm: bias_pk[:, i] for i, nm in enumerate(BIAS_NAMES)}
            geneT_sb = acts_sb[:, 0]
            bionT_sb = acts_sb[:, 1]

            # ---- weights (sync): fc+wqk early, wvo late ----
            wfc_sb = wts.tile([128, 2, KCF, HID], ct)
            nc.sync.dma_start(wfc_sb[:], d_wfc[:])
            wqk_sb = wts.tile([128, 4, NCH, HID], ct)
            nc.sync.dma_start(wqk_sb[:], d_wqk[:])

            # denseT on the scalar-engine HWDGE queue (parallel descriptor gen)
            dT_sb = []
            for i in range(4):
                t = dns.tile([128, 8, NCH, 128], ct, tag=f"dT{i}")
                nc.scalar.dma_start(t[:], d_denseT[:, 8 * i:8 * (i + 1)])
                dT_sb.append(t)

            # natural dense (sync), 2 chunks
            dn_sb = []
            for i in range(2):
                t = dns.tile([128, 16, HID], ct, tag=f"dn{i}")
                nc.sync.dma_start(t[:], d_dense[:, 16 * i:16 * (i + 1)])
                dn_sb.append(t)

            wvo_sb = wts.tile([128, 4, NCH, HID], ct)
            nc.sync.dma_start(wvo_sb[:], d_wvo[:])

            w_sb = {
                "fc0_w": wfc_sb[:, 0], "fc1_w": wfc_sb[:, 1],
                "wq0": wqk_sb[:, 0], "wq1": wqk_sb[:, 1],
                "wk0T": wqk_sb[:, 2], "wk1T": wqk_sb[:, 3],
                "wv0": wvo_sb[:, 0], "wv1": wvo_sb[:, 1],
                "wo0": wvo_sb[:, 2], "wo1": wvo_sb[:, 3],
            }

            def dview(b):
                return dn_sb[b // 16][:, b % 16, :]

            def dTview(b):
                return dT_sb[b // 8][:, b % 8]

            # ---- phase A: activationsT chains (features on partitions) ----
            def chain(dst, w, kcn, rhs, bias=None, func=Iden):
                for mc in range(NCH):
                    pt = ps.tile([128, G], f32, tag="mm32", bufs=3)
                    for kc in range(kcn):
                        nc.tensor.matmul(
                            pt[:],
                            w[:, kc, mc * 128:(mc + 1) * 128],
                            rhs[:, kc, :],
                            start=(kc == 0),
                            stop=(kc == kcn - 1),
                        )
                    bb = bias[:, mc:mc + 1] if bias is not None else 0.0
                    nc.scalar.activation(dst[:, mc, :], pt[:], func, bias=bb, scale=1.0)

            gene_pT = wrk.tile([128, NCH, G], ct)
            chain(gene_pT, w_sb["fc0_w"], KCF, geneT_sb, bias_sb["fc0_b"], Relu)
            bion_pT = wrk.tile([128, NCH, G], ct)
            chain(bion_pT, w_sb["fc1_w"], KCF, bionT_sb, bias_sb["fc1_b"], Relu)
            y0T = wrk.tile([128, NCH, G], ct)
            chain(y0T, w_sb["wq0"], NCH, gene_pT, bias_sb["bq0"], Iden)
            y1T = wrk.tile([128, NCH, G], ct)
            chain(y1T, w_sb["wq1"], NCH, bion_pT, bias_sb["bq1"], Iden)

            # u vectors, interleaved (graph-major, branch minor) [128, NCH, R]
            uT = wrk.tile([128, NCH, R], ct)
            for mc in range(NCH):
                usl = uT[:, mc, :].rearrange("p (g q) -> p q g", q=2)
                for q, (wk, yT) in enumerate((("wk0T", y0T), ("wk1T", y1T))):
                    pt = ps.tile([128, G], f32, tag="mm32", bufs=3)
                    for kc in range(NCH):
                        nc.tensor.matmul(
                            pt[:],
                            w_sb[wk][:, kc, mc * 128:(mc + 1) * 128],
                            yT[:, kc, :],
                            start=(kc == 0),
                            stop=(kc == NCH - 1),
                        )
                    nc.vector.tensor_copy(usl[:, q, :], pt[:])

            # c scalars: c_q[b] = q'_b · qk_q  -> [1, R] interleaved
            c_int = wrk.tile([1, R], ct)
            csl = c_int[0:1, :].rearrange("p (g q) -> p q g", q=2)
            for q, aT in enumerate((gene_pT, bion_pT)):
                pt = ps.tile([1, G], f32, tag="mm32", bufs=3)
                for kc in range(NCH):
                    nc.tensor.matmul(
                        pt[:],
                        qks_sb[:, q, kc:kc + 1],
                        aT[:, kc, :],
                        start=(kc == 0),
                        stop=(kc == NCH - 1),
                    )
                nc.vector.tensor_copy(csl[:, q, :], pt[:])

            # ---- phase B: e_ps[node, 2b+q] = dense_b @ u_bq + c_bq ----
            # c broadcast first (start=True clears the whole tile), then
            # per-graph accumulating matvecs against host-transposed denseT.
            e_ps = ps.tile([128, R], f32, tag="eacc")
            nc.tensor.matmul(
                e_ps[:], ones_ct[:], c_int[:], start=True, stop=False,
                skip_group_check=True,
            )
            for b in range(G):
                dT = dTview(b)
                for mc in range(NCH):
                    nc.tensor.matmul(
                        e_ps[:, 2 * b:2 * b + 2],
                        dT[:, mc, :],
                        uT[:, mc, 2 * b:2 * b + 2],
                        start=False,
                        stop=(b == G - 1 and mc == NCH - 1),
                        skip_group_check=True,
                    )

            # ---- softmax over nodes (batched, rows = 2b+q) ----
            e_sb = wrk.tile([128, R], f32)
            nc.scalar.activation(e_sb[:], e_ps[:], Copy, bias=0.0, scale=1.0 / SCALE)
            etp = ps.tile([R, NPG], f32, tag="tp", bufs=2)
            nc.tensor.transpose(etp[:], e_sb[:], ident_f32[:])
            em = wrk.tile([R, NPG], f32)
            nc.vector.tensor_add(em[:], etp[:], mask_sb[:])
            nmax = wrk.tile([R, 1], f32)
            nc.vector.reduce_max(nmax[:], em[:], axis=AX, negate=True)
            p_sb = wrk.tile([R, NPG], f32)
            rsum = wrk.tile([R, 1], f32)
            nc.scalar.activation(
                p_sb[:], em[:], Exp, bias=nmax[:, 0:1], scale=1.0, accum_out=rsum[:]
            )
            rinv = wrk.tile([R, 1], f32)
            nc.vector.reciprocal(rinv[:], rsum[:])
            attn = wrk.tile([R, NPG], ct)
            nc.vector.tensor_scalar_mul(attn[:], p_sb[:], rinv[:, 0:1])
            atp = ps.tile([128, R], ct, tag="tp", bufs=2)
            nc.tensor.transpose(atp[:], attn[:], ident_ct[0:R, 0:R])
            attnT = wrk.tile([128, R], ct)
            nc.vector.tensor_copy(attnT[:], atp[:])

            # ---- h: h_ps[feat_chunk, mc*R + 2b+q] = attn_{b,q} @ dense_b ----
            h_ps = ps.tile([128, NCH * R], f32, tag="hacc")
            for b in range(G):
                dv = dview(b)
                for mc in range(NCH):
                    nc.tensor.matmul(
                        h_ps[:, mc * R + 2 * b: mc * R + 2 * b + 2],
                        dv[:, mc * 128:(mc + 1) * 128],
                        attnT[:, 2 * b:2 * b + 2],
                        start=True,
                        stop=True,
                    )
            h0T = wrk.tile([128, NCH, G], ct)
            h1T = wrk.tile([128, NCH, G], ct)
            for mc in range(NCH):
                hsl = h_ps[:, mc * R:(mc + 1) * R].rearrange("p (g q) -> p q g", q=2)
                nc.vector.tensor_copy(h0T[:, mc, :], hsl[:, 0, :])
                nc.scalar.copy(h1T[:, mc, :], hsl[:, 1, :])

            # ---- phase C: out = (h0 wv0 + bv0) wo0 + (h1 wv1 + bv1) wo1 + cvec ----
            z0T = wrk.tile([128, NCH, G], ct)
            chain(z0T, w_sb["wv0"], NCH, h0T, bias_sb["bv0"], Iden)
            z1T = wrk.tile([128, NCH, G], ct)
            chain(z1T, w_sb["wv1"], NCH, h1T, bias_sb["bv1"], Iden)

            out_sb = wrk.tile([G, HID], f32)
            for half in range(2):
                nsl = slice(half * 384, (half + 1) * 384)
                po = ps.tile([G, 384], f32, tag="tp", bufs=2)
                for kc in range(NCH):
                    nc.tensor.matmul(
                        po[:], z0T[:, kc, :], w_sb["wo0"][:, kc, nsl],
                        start=(kc == 0), stop=False,
                    )
                for kc in range(NCH):
                    nc.tensor.matmul(
                        po[:], z1T[:, kc, :], w_sb["wo1"][:, kc, nsl],
                        start=False, stop=False,
                    )
                nc.tensor.matmul(
                    po[:], ones_ct[0:1, 0:G], cvec_sb[0:1, nsl],
                    start=False, stop=True,
                )
                nc.vector.tensor_copy(out_sb[:, nsl], po[:])
            nc.sync.dma_start(d_out[:], out_sb[:])

    nc.compile()
    return nc


def _get_nc():
    global _NC
    if _NC is None:
        _NC = _build_nc()
    return _NC


def _pm(w, kc):
    """[kc*128, N] -> partition-major [128, kc, N]."""
    return np.ascontiguousarray(w.reshape(kc, 128, -1).transpose(1, 0, 2))


def _prep_in_maps(inputs):
    x = np.asarray(inputs["x"], dtype=np.float32)
    batch = np.asarray(inputs["batch"]).astype(np.int64)
    gene = np.asarray(inputs["gene"], dtype=np.float32)
    bionic = np.asarray(inputs["bionic"], dtype=np.float32)
    N = x.shape[0]

    counts = np.bincount(batch, minlength=B)[:B]
    starts = np.zeros(B, np.int64)
    np.cumsum(counts[:-1], out=starts[1:])
    pos = np.arange(N, dtype=np.int64) - starts[batch]
    ok = pos < NPG
    dense = np.zeros((B, NPG, HID), np.float32)
    dense[batch[ok], pos[ok]] = x[ok]
    dense_bf = dense.astype(BF16)
    mask = np.where(
        np.arange(NPG)[None, :] < np.minimum(counts, NPG)[:, None], 0.0, NEG
    ).astype(np.float32)

    f32 = lambda k: np.asarray(inputs[k], dtype=np.float32)
    wfc = np.stack([_pm(f32("fc0_w").astype(BF16), KCF),
                    _pm(f32("fc1_w").astype(BF16), KCF)], axis=1)
    wqk = np.stack([
        _pm(f32("a0_wq").astype(BF16), NCH),
        _pm(f32("a1_wq").astype(BF16), NCH),
        _pm(np.ascontiguousarray(f32("a0_wk").T).astype(BF16), NCH),
        _pm(np.ascontiguousarray(f32("a1_wk").T).astype(BF16), NCH),
    ], axis=1)
    wvo = np.stack([
        _pm(f32("a0_wv").astype(BF16), NCH),
        _pm(f32("a1_wv").astype(BF16), NCH),
        _pm(f32("a0_wo").astype(BF16), NCH),
        _pm(f32("a1_wo").astype(BF16), NCH),
    ], axis=1)
    bias = np.stack(
        [f32(k).reshape(NCH, 128).T for k in
         ("fc0_b", "fc1_b", "a0_bq", "a1_bq", "a0_bv", "a1_bv")], axis=1)
    qks = np.stack([
        (f32("a0_wq") @ f32("a0_bk")).astype(BF16).reshape(NCH, 128).T,
        (f32("a1_wq") @ f32("a1_bk")).astype(BF16).reshape(NCH, 128).T,
    ], axis=1)
    cvec = (f32("a0_bo") + f32("a1_bo")).astype(BF16).reshape(1, HID)
    s0 = float(f32("a0_bq") @ f32("a0_bk"))
    s1 = float(f32("a1_bq") @ f32("a1_bk"))

    w = {
        "wfc": np.ascontiguousarray(wfc),
        "wqk": np.ascontiguousarray(wqk),
        "wvo": np.ascontiguousarray(wvo),
        "bias": np.ascontiguousarray(bias),
        "qks": np.ascontiguousarray(qks),
        "cvec": np.ascontiguousarray(cvec),
    }

    in_maps = []
    for j in range(NCORES):
        g0, g1 = j * G, (j + 1) * G
        m = np.repeat(mask[g0:g1], 2, axis=0)
        m[0::2] += s0 / SCALE
        m[1::2] += s1 / SCALE
        dc = dense_bf[g0:g1]  # [G, NPG, HID]
        im = {
            # [128 node-partitions, G, HID]
            "dense": np.ascontiguousarray(dc.transpose(1, 0, 2)),
            # [128 feat-partitions, G, NCH, 128 nodes]
            "denseT": np.ascontiguousarray(
                dc.reshape(G, NPG, NCH, 128).transpose(3, 0, 2, 1)
            ),
            "maskadd": np.ascontiguousarray(m),
            "acts": np.ascontiguousarray(np.stack([
                gene[g0:g1].astype(BF16).reshape(G, KCF, 128).transpose(2, 1, 0),
                bionic[g0:g1].astype(BF16).reshape(G, KCF, 128).transpose(2, 1, 0),
            ], axis=1)),
        }
        im.update(w)
        in_maps.append(im)
    return in_maps


def run(inputs, trace=False):
    _install_ntff_hook()
    from concourse.bass_utils import run_bass_kernel_spmd

    nc = _get_nc()
    in_maps = _prep_in_maps(inputs)
    res = run_bass_kernel_spmd(
        nc, in_maps, core_ids=list(range(NCORES)), trace=trace
    )
    out = np.concatenate([res.results[i]["out"] for i in range(NCORES)], axis=0)
    return out.astype(np.float32), res


def kernel(**inputs):
    return run(inputs, trace=False)[0]
